# revision 3
# baseline (speedup 1.0000x reference)
"""Ensemble detection fusion (weighted-boxes-fusion match + soft-NMS dedup)
for Trainium2, 8 NeuronCores.

Strategy: the O(N^2) work — 4096x2048 greedy-match IoU tests and the
6144x6144 NMS IoU tests — runs on-device as a conservative *filter*
(division-free IoU threshold test with a relative margin), sharded row-wise
across the 8 cores.  Matches/suppressions at these thresholds (0.8 / 0.95
IoU) are extremely sparse, so the host then *rescues* only the flagged rows
with the exact reference arithmetic (bitwise-matching fp32 op order) and
resolves the short sequential dependency chains (greedy 'used' set, NMS
suppression scan) on those few rows.

Device test per pair (q, t):  inter(q,t) >= (1-MU)*c*(area_q + area_t)
with c = thr/(1+thr), which in exact arithmetic equals iou >= thr*(1-ish);
the MU margin makes the filter a strict superset of the exact test.
"""

import numpy as np

N1, N2 = 4096, 2048
NB = N1 + N2
CORES = 8
YOLO_W = 0.5
FRCNN_W = 0.5
MATCH_IOU = 0.8
NMS_IOU = 0.95
MU = 1e-3  # relative margin on the device filter (superset guarantee)

QA_PT = N1 // CORES // 128  # 4 query tiles of 128 rows per core (stage A)
QB_PT = NB // CORES // 128  # 6 query tiles per core (stage B)
CHUNK_A = 2048
CHUNK_B = 1536
NCH_A = N2 // CHUNK_A   # 1
NCH_B = NB // CHUNK_B   # 4

_PROGRAM_CACHE = {}


def _build_stage(nc, tc, tile, mybir, q_dram, t_dram, out_dram, nt, chunk, qtiles, tag):
    f32 = mybir.dt.float32
    Alu = mybir.AluOpType
    Act = mybir.ActivationFunctionType
    nch = nt // chunk
    with (
        tc.tile_pool(name=f"tgt{tag}", bufs=1) as pt,
        tc.tile_pool(name=f"q{tag}", bufs=2) as pq,
        tc.tile_pool(name=f"w{tag}", bufs=2) as pw,
        tc.tile_pool(name=f"c{tag}", bufs=4) as pc,
    ):
        tfull = pt.tile([128, 5 * nt], f32, tag="tfull")
        bcast_blk = 2048
        for off in range(0, 5 * nt, bcast_blk):
            blk = min(bcast_blk, 5 * nt - off)
            trow = pq.tile([1, bcast_blk], f32, tag="trow")
            nc.sync.dma_start(trow[:1, :blk], t_dram.ap()[:, off : off + blk])
            nc.gpsimd.partition_broadcast(
                tfull[:, off : off + blk], trow[0:1, :blk]
            )
        TX1 = tfull[:, 0 * nt : 1 * nt]
        TY1 = tfull[:, 1 * nt : 2 * nt]
        TX2 = tfull[:, 2 * nt : 3 * nt]
        TY2 = tfull[:, 3 * nt : 4 * nt]
        TS = tfull[:, 4 * nt : 5 * nt]
        for qt in range(qtiles):
            q = pq.tile([128, 5], f32, tag="q")
            nc.sync.dma_start(q[:, :], q_dram.ap()[qt, :, :])
            qx1 = q[:, 0:1]
            qy1 = q[:, 1:2]
            qx2 = q[:, 2:3]
            qy2 = q[:, 3:4]
            qs = q[:, 4:5]
            for ch in range(nch):
                sl = slice(ch * chunk, (ch + 1) * chunk)
                a1 = pw.tile([128, chunk], f32, tag="acc")
                nc.vector.tensor_scalar(a1[:, :], TX1[:, sl], qx1, None, Alu.max)
                dx = pw.tile([128, chunk], f32, tag="d")
                nc.vector.scalar_tensor_tensor(
                    dx[:, :], TX2[:, sl], qx2, a1[:, :],
                    op0=Alu.min, op1=Alu.subtract,
                )
                a2 = pw.tile([128, chunk], f32, tag="acc")
                nc.vector.tensor_scalar(a2[:, :], TY1[:, sl], qy1, None, Alu.max)
                dy = pw.tile([128, chunk], f32, tag="d")
                nc.vector.scalar_tensor_tensor(
                    dy[:, :], TY2[:, sl], qy2, a2[:, :],
                    op0=Alu.min, op1=Alu.subtract,
                )
                dxr = pw.tile([128, chunk], f32, tag="r")
                nc.scalar.activation(dxr[:, :], dx[:, :], Act.Relu)
                dyr = pw.tile([128, chunk], f32, tag="r")
                nc.scalar.activation(dyr[:, :], dy[:, :], Act.Relu)
                p = pw.tile([128, chunk], f32, tag="p")
                nc.vector.tensor_tensor(p[:, :], dxr[:, :], dyr[:, :], Alu.mult)
                g = pw.tile([128, chunk], f32, tag="g")
                cnt = pc.tile([128, 1], f32, tag="cnt")
                nc.vector.scalar_tensor_tensor(
                    g[:, :], p[:, :], qs, TS[:, sl],
                    op0=Alu.subtract, op1=Alu.is_ge, accum_out=cnt[:, :],
                )
                nc.sync.dma_start(out_dram.ap()[qt, ch, :, :], cnt[:, :])


def _build_program():
    import concourse.bacc as bacc
    import concourse.mybir as mybir
    from concourse import tile

    f32 = mybir.dt.float32
    nc = bacc.Bacc(
        "TRN2", target_bir_lowering=False, debug=False, num_devices=CORES
    )
    qa = nc.dram_tensor("qa", [QA_PT, 128, 5], f32, kind="ExternalInput")
    qb = nc.dram_tensor("qb", [QB_PT, 128, 5], f32, kind="ExternalInput")
    ta = nc.dram_tensor("ta", [1, 5 * N2], f32, kind="ExternalInput")
    tb = nc.dram_tensor("tb", [1, 5 * NB], f32, kind="ExternalInput")
    outa = nc.dram_tensor("outa", [QA_PT, NCH_A, 128, 1], f32, kind="ExternalOutput")
    outb = nc.dram_tensor("outb", [QB_PT, NCH_B, 128, 1], f32, kind="ExternalOutput")

    with tile.TileContext(nc) as tc:
        _build_stage(nc, tc, tile, mybir, qa, ta, outa, N2, CHUNK_A, QA_PT, "A")
        _build_stage(nc, tc, tile, mybir, qb, tb, outb, NB, CHUNK_B, QB_PT, "B")
    nc.compile()
    return nc


def get_program():
    if "nc" not in _PROGRAM_CACHE:
        _PROGRAM_CACHE["nc"] = _build_program()
    return _PROGRAM_CACHE["nc"]


def _iou_row(box, B):
    # Exact replica of reference _iou_one_vs_many op order (fp32, IEEE).
    x1 = np.maximum(box[0], B[:, 0])
    y1 = np.maximum(box[1], B[:, 1])
    x2 = np.minimum(box[2], B[:, 2])
    y2 = np.minimum(box[3], B[:, 3])
    inter = np.maximum(x2 - x1, np.float32(0.0)) * np.maximum(y2 - y1, np.float32(0.0))
    a1 = (box[2] - box[0]) * (box[3] - box[1])
    a2 = (B[:, 2] - B[:, 0]) * (B[:, 3] - B[:, 1])
    return inter / (a1 + a2 - inter)


def make_device_inputs(b1, b2, a1, a2, ball, aball):
    cA = np.float32((1.0 - MU) * MATCH_IOU / (1.0 + MATCH_IOU))
    cB = np.float32((1.0 - MU) * NMS_IOU / (1.0 + NMS_IOU))
    qa_all = np.concatenate([b1, (cA * a1)[:, None]], axis=1).astype(np.float32)
    qb_all = np.concatenate([ball, (cB * aball)[:, None]], axis=1).astype(np.float32)
    ta = np.concatenate(
        [b2[:, 0], b2[:, 1], b2[:, 2], b2[:, 3], cA * a2]
    ).astype(np.float32).reshape(1, -1)
    tb = np.concatenate(
        [ball[:, 0], ball[:, 1], ball[:, 2], ball[:, 3], cB * aball]
    ).astype(np.float32).reshape(1, -1)
    ta = np.ascontiguousarray(ta)
    tb = np.ascontiguousarray(tb)
    in_maps = []
    ra = N1 // CORES
    rb = NB // CORES
    for c in range(CORES):
        in_maps.append(
            {
                "qa": np.ascontiguousarray(
                    qa_all[c * ra : (c + 1) * ra].reshape(QA_PT, 128, 5)
                ),
                "qb": np.ascontiguousarray(
                    qb_all[c * rb : (c + 1) * rb].reshape(QB_PT, 128, 5)
                ),
                "ta": ta,
                "tb": tb,
            }
        )
    return in_maps


def kernel(**inputs):
    yolo_boxes = np.asarray(inputs["yolo_boxes"], dtype=np.float32)
    yolo_scores = np.asarray(inputs["yolo_scores"], dtype=np.float32)
    yolo_labels = np.asarray(inputs["yolo_labels"], dtype=np.int32)
    frcnn_boxes = np.asarray(inputs["frcnn_boxes"], dtype=np.float32)
    frcnn_scores = np.asarray(inputs["frcnn_scores"], dtype=np.float32)
    frcnn_labels = np.asarray(inputs["frcnn_labels"], dtype=np.int32)
    h = float(np.asarray(inputs["h"]))
    w = float(np.asarray(inputs["w"]))

    wh = np.array([w, h, w, h], dtype=np.float32)
    b1 = (yolo_boxes / wh).astype(np.float32)
    b2 = (frcnn_boxes / wh).astype(np.float32)
    s1 = (yolo_scores * np.float32(YOLO_W)).astype(np.float32)
    s2 = (frcnn_scores * np.float32(FRCNN_W)).astype(np.float32)
    l1, l2 = yolo_labels, frcnn_labels

    a1 = (b1[:, 2] - b1[:, 0]) * (b1[:, 3] - b1[:, 1])
    a2 = (b2[:, 2] - b2[:, 0]) * (b2[:, 3] - b2[:, 1])
    ball = np.concatenate([b1, b2], axis=0)
    aball = np.concatenate([a1, a2], axis=0)

    # --- device filter: 8-core SPMD sweep over both IoU-test matrices ---
    from concourse.bass_utils import run_bass_kernel_spmd

    nc = get_program()
    in_maps = make_device_inputs(b1, b2, a1, a2, ball, aball)
    import time as _time

    _t0 = _time.time()
    res = run_bass_kernel_spmd(nc, in_maps, core_ids=list(range(CORES)))
    _PROGRAM_CACHE["device_wall_ns"] = int((_time.time() - _t0) * 1e9)
    if getattr(res, "exec_time_ns", None) is not None:
        _PROGRAM_CACHE["exec_time_ns"] = res.exec_time_ns
    cnt_a = np.concatenate(
        [res.results[c]["outa"].reshape(-1) for c in range(CORES)]
    )
    cnt_b = np.concatenate(
        [res.results[c]["outb"].sum(axis=1).reshape(-1) for c in range(CORES)]
    )
    flags_a = np.nonzero(cnt_a > 0.5)[0]
    # every stage-B row passes against itself (diagonal), so > 1 means a
    # real (off-diagonal) near-duplicate candidate
    flags_b = np.nonzero(cnt_b > 1.5)[0]

    return _host_resolve(
        b1, b2, s1, s2, l1, l2, wh, flags_a, flags_b
    )


def _host_resolve(b1, b2, s1, s2, l1, l2, wh, flags_a, flags_b):
    # --- exact greedy match on candidate rows only ---
    used = np.zeros(N2, dtype=bool)
    mboxes = b1.copy()
    mscores = s1.copy()
    merged_rows = []
    thr_match = np.float32(MATCH_IOU)
    for i in flags_a:
        iou = _iou_row(b1[i], b2)
        iou = np.where((~used) & (l2 == l1[i]), iou, np.float32(0.0))
        j = int(np.argmax(iou))
        if iou[j] >= thr_match:
            tot = s1[i] + s2[j]
            mboxes[i] = (b1[i] * s1[i] + b2[j] * s2[j]) / tot
            mscores[i] = tot
            used[j] = True
            merged_rows.append(int(i))

    boxes_all = np.concatenate([mboxes, b2], axis=0)
    scores_all = np.concatenate([mscores, s2], axis=0)
    labels_all = np.concatenate([l1, l2], axis=0)
    valid = np.concatenate([np.ones(N1, dtype=bool), ~used])

    key = np.where(valid, scores_all, np.float32(-1.0))
    order = np.argsort(-key, kind="stable")
    bs = boxes_all[order]
    ss = scores_all[order]
    ls = labels_all[order]
    vs = valid[order]
    inv = np.empty(NB, dtype=np.int64)
    inv[order] = np.arange(NB)

    # --- exact NMS edge rescue ---
    # device stage B ran on PRE-merge boxes: flagged rows cover all edges
    # between unmerged boxes (symmetric); merged rows are rescanned fully.
    r_unsorted = set(int(x) for x in flags_b) | set(merged_rows)
    thr_nms = np.float32(NMS_IOU)
    edges = {}
    for r in r_unsorted:
        p = int(inv[r])
        iou = _iou_row(bs[p], bs)
        hits = np.nonzero((iou >= thr_nms) & (ls == ls[p]))[0]
        for qq in hits:
            qq = int(qq)
            if qq == p:
                continue
            lo, hi = (p, qq) if p < qq else (qq, p)
            edges.setdefault(lo, set()).add(hi)

    # --- sequential suppression scan (only edge nodes matter) ---
    suppressed = ~vs
    keep = np.zeros(NB, dtype=bool)
    if edges:
        for i in range(NB):
            k = not suppressed[i]
            keep[i] = k
            if k and i in edges:
                for b_ in edges[i]:
                    suppressed[b_] = True
    else:
        keep = vs.copy()

    label_map = np.array([2, 1], dtype=np.int32)
    boxes_out = (bs * wh).astype(np.float32)
    labels_out = label_map[np.clip(ls, 0, 1)].astype(np.int32)
    return boxes_out, labels_out, ss.astype(np.float32), keep


# revision 6
# speedup vs baseline: 1.1034x; 1.1034x over previous
"""Ensemble detection fusion (weighted-boxes-fusion match + soft-NMS dedup)
for Trainium2, 8 NeuronCores.

Strategy: the O(N^2) work — 4096x2048 greedy-match IoU tests and the
6144x6144 NMS IoU tests — runs on-device as a conservative *filter*
(division-free IoU threshold test with safety margins), sharded row-wise
across the 8 cores.  Matches/suppressions at these thresholds (0.8 / 0.95
IoU) are extremely sparse, so the host then *rescues* only the flagged rows
with the exact reference arithmetic (fp32, matching op order) and resolves
the short sequential dependency chains (greedy 'used' set, NMS suppression
scan) on those few rows.

Device test per pair (q, t), in fp16 pixel space:
    (relu(dx)+K) * (relu(dy)+K)  >=  c*(1-MU)*(area_q + area_t)
with c = thr/(1+thr).  In exact arithmetic  inter >= c*(Aq+At)  <=>
iou >= thr; the +K (2 px) additive slack covers fp16 coordinate rounding
(<=1px absolute on dx/dy) and MU covers all relative rounding, so the
device flag set is a strict superset of the exact >=thr set.  False
positives only cost a few extra host-rescued rows.
"""

import numpy as np

N1, N2 = 4096, 2048
NB = N1 + N2
CORES = 8
YOLO_W = 0.5
FRCNN_W = 0.5
MATCH_IOU = 0.8
NMS_IOU = 0.95
MU = 0.02   # relative margin on the device filter
KPX = 2.0   # additive pixel slack on clamped overlap widths

QA_PT = N1 // CORES // 128  # 4 query tiles of 128 rows per core (stage A)
QB_PT = NB // CORES // 128  # 6 query tiles per core (stage B)
CHUNK_A = 2048
CHUNK_B = 3072
NCH_A = N2 // CHUNK_A   # 1
NCH_B = NB // CHUNK_B   # 2

# target blob layout (fp16):  x1[NB] y1[NB] x2[NB] y2[NB] sB[NB] sA[N2]
# stage B uses blocks 0-4; stage A uses the b2 tail (rows N1..NB) of the
# coordinate blocks plus the sA block.
TLEN = 5 * NB + N2

_PROGRAM_CACHE = {}


def _emit_combo(nc, mybir, TX1, TY1, TX2, TY2, TS, q, pw, pc, out_ap, chunk):
    f16 = mybir.dt.float16
    f32 = mybir.dt.float32
    Alu = mybir.AluOpType
    Act = mybir.ActivationFunctionType
    qx1, qy1, qx2, qy2, qs = (q[:, k : k + 1] for k in range(5))
    a1 = pw.tile([128, chunk], f16, tag="acc")
    nc.vector.tensor_scalar(a1[:, :], TX1, qx1, None, Alu.max)
    dx = pw.tile([128, chunk], f16, tag="d")
    nc.vector.scalar_tensor_tensor(
        dx[:, :], TX2, qx2, a1[:, :], op0=Alu.min, op1=Alu.subtract
    )
    a2 = pw.tile([128, chunk], f16, tag="acc")
    nc.vector.tensor_scalar(a2[:, :], TY1, qy1, None, Alu.max)
    dy = pw.tile([128, chunk], f16, tag="d")
    nc.vector.scalar_tensor_tensor(
        dy[:, :], TY2, qy2, a2[:, :], op0=Alu.min, op1=Alu.subtract
    )
    dxk = pw.tile([128, chunk], f16, tag="r")
    nc.vector.tensor_scalar(
        dxk[:, :], dx[:, :], 0.0, KPX, Alu.max, Alu.add
    )
    dyk = pw.tile([128, chunk], f16, tag="k")
    nc.vector.tensor_scalar(
        dyk[:, :], dy[:, :], 0.0, KPX, Alu.max, Alu.add
    )
    p = pw.tile([128, chunk], f16, tag="p")
    nc.vector.tensor_tensor(p[:, :], dxk[:, :], dyk[:, :], Alu.mult)
    g = pw.tile([128, chunk], f16, tag="g")
    cnt = pc.tile([128, 1], f32, tag="cnt")
    nc.vector.scalar_tensor_tensor(
        g[:, :], p[:, :], qs, TS,
        op0=Alu.subtract, op1=Alu.is_ge, accum_out=cnt[:, :],
    )
    nc.sync.dma_start(out_ap, cnt[:, :])


def _build_program():
    import concourse.bacc as bacc
    import concourse.mybir as mybir
    from concourse import tile

    f16 = mybir.dt.float16
    f32 = mybir.dt.float32
    nc = bacc.Bacc(
        "TRN2", target_bir_lowering=False, debug=False, num_devices=CORES
    )
    qa = nc.dram_tensor("qa", [QA_PT, 128, 5], f32, kind="ExternalInput")
    qb = nc.dram_tensor("qb", [QB_PT, 128, 5], f32, kind="ExternalInput")
    tbl = nc.dram_tensor("tbl", [1, TLEN], f16, kind="ExternalInput")
    outa = nc.dram_tensor("outa", [QA_PT, NCH_A, 128, 1], f32, kind="ExternalOutput")
    outb = nc.dram_tensor("outb", [QB_PT, NCH_B, 128, 1], f32, kind="ExternalOutput")

    with tile.TileContext(nc) as tc:
        with (
            tc.tile_pool(name="tgt", bufs=1) as pt,
            tc.tile_pool(name="qs", bufs=3) as pq,
            tc.tile_pool(name="wk", bufs=2) as pw,
            tc.tile_pool(name="ct", bufs=4) as pc,
        ):
            tfull = pt.tile([128, TLEN], f16, tag="tfull")
            blk = 4096
            for off in range(0, TLEN, blk):
                n = min(blk, TLEN - off)
                trow = pq.tile([1, blk], f16, tag="trow")
                nc.sync.dma_start(trow[:1, :n], tbl.ap()[:, off : off + n])
                nc.gpsimd.partition_broadcast(
                    tfull[:, off : off + n], trow[0:1, :n]
                )
            X1 = tfull[:, 0 * NB : 1 * NB]
            Y1 = tfull[:, 1 * NB : 2 * NB]
            X2 = tfull[:, 2 * NB : 3 * NB]
            Y2 = tfull[:, 3 * NB : 4 * NB]
            SB = tfull[:, 4 * NB : 5 * NB]
            SA = tfull[:, 5 * NB : 5 * NB + N2]

            # stage A: queries = yolo shard, targets = frcnn boxes
            # (= rows N1..NB of the coordinate blocks)
            for qt in range(QA_PT):
                q = pq.tile([128, 5], f32, tag="q")
                nc.sync.dma_start(q[:, :], qa.ap()[qt, :, :])
                for ch in range(NCH_A):
                    sl = slice(N1 + ch * CHUNK_A, N1 + (ch + 1) * CHUNK_A)
                    sa = slice(ch * CHUNK_A, (ch + 1) * CHUNK_A)
                    _emit_combo(
                        nc, mybir,
                        X1[:, sl], Y1[:, sl], X2[:, sl], Y2[:, sl], SA[:, sa],
                        q, pw, pc, outa.ap()[qt, ch, :, :], CHUNK_A,
                    )
            # stage B: queries = all-boxes shard, targets = all boxes
            for qt in range(QB_PT):
                q = pq.tile([128, 5], f32, tag="q")
                nc.sync.dma_start(q[:, :], qb.ap()[qt, :, :])
                for ch in range(NCH_B):
                    sl = slice(ch * CHUNK_B, (ch + 1) * CHUNK_B)
                    _emit_combo(
                        nc, mybir,
                        X1[:, sl], Y1[:, sl], X2[:, sl], Y2[:, sl], SB[:, sl],
                        q, pw, pc, outb.ap()[qt, ch, :, :], CHUNK_B,
                    )
    nc.compile()
    return nc


def get_program():
    if "nc" not in _PROGRAM_CACHE:
        _PROGRAM_CACHE["nc"] = _build_program()
    return _PROGRAM_CACHE["nc"]


def _iou_row(box, B):
    # Exact replica of reference _iou_one_vs_many op order (fp32, IEEE).
    x1 = np.maximum(box[0], B[:, 0])
    y1 = np.maximum(box[1], B[:, 1])
    x2 = np.minimum(box[2], B[:, 2])
    y2 = np.minimum(box[3], B[:, 3])
    inter = np.maximum(x2 - x1, np.float32(0.0)) * np.maximum(y2 - y1, np.float32(0.0))
    a1 = (box[2] - box[0]) * (box[3] - box[1])
    a2 = (B[:, 2] - B[:, 0]) * (B[:, 3] - B[:, 1])
    return inter / (a1 + a2 - inter)


def make_device_inputs(p1, p2, pall):
    """p1/p2/pall: pixel-space fp32 box arrays."""
    a1 = (p1[:, 2] - p1[:, 0]) * (p1[:, 3] - p1[:, 1])
    a2 = (p2[:, 2] - p2[:, 0]) * (p2[:, 3] - p2[:, 1])
    aall = np.concatenate([a1, a2])
    cA = np.float32((1.0 - MU) * MATCH_IOU / (1.0 + MATCH_IOU))
    cB = np.float32((1.0 - MU) * NMS_IOU / (1.0 + NMS_IOU))
    qa_all = np.concatenate([p1, (cA * a1)[:, None]], axis=1).astype(np.float32)
    qb_all = np.concatenate([pall, (cB * aall)[:, None]], axis=1).astype(np.float32)
    tbl = np.concatenate(
        [pall[:, 0], pall[:, 1], pall[:, 2], pall[:, 3], cB * aall, cA * a2]
    ).astype(np.float16).reshape(1, -1)
    tbl = np.ascontiguousarray(tbl)
    in_maps = []
    ra = N1 // CORES
    rb = NB // CORES
    for c in range(CORES):
        in_maps.append(
            {
                "qa": np.ascontiguousarray(
                    qa_all[c * ra : (c + 1) * ra].reshape(QA_PT, 128, 5)
                ),
                "qb": np.ascontiguousarray(
                    qb_all[c * rb : (c + 1) * rb].reshape(QB_PT, 128, 5)
                ),
                "tbl": tbl,
            }
        )
    return in_maps


def kernel(**inputs):
    yolo_boxes = np.asarray(inputs["yolo_boxes"], dtype=np.float32)
    yolo_scores = np.asarray(inputs["yolo_scores"], dtype=np.float32)
    yolo_labels = np.asarray(inputs["yolo_labels"], dtype=np.int32)
    frcnn_boxes = np.asarray(inputs["frcnn_boxes"], dtype=np.float32)
    frcnn_scores = np.asarray(inputs["frcnn_scores"], dtype=np.float32)
    frcnn_labels = np.asarray(inputs["frcnn_labels"], dtype=np.int32)
    h = float(np.asarray(inputs["h"]))
    w = float(np.asarray(inputs["w"]))

    wh = np.array([w, h, w, h], dtype=np.float32)
    b1 = (yolo_boxes / wh).astype(np.float32)
    b2 = (frcnn_boxes / wh).astype(np.float32)
    s1 = (yolo_scores * np.float32(YOLO_W)).astype(np.float32)
    s2 = (frcnn_scores * np.float32(FRCNN_W)).astype(np.float32)
    l1, l2 = yolo_labels, frcnn_labels

    # --- device filter: 8-core SPMD sweep over both IoU-test matrices ---
    from concourse.bass_utils import run_bass_kernel_spmd

    nc = get_program()
    pall = np.concatenate([yolo_boxes, frcnn_boxes], axis=0)
    in_maps = make_device_inputs(yolo_boxes, frcnn_boxes, pall)
    import time as _time

    _t0 = _time.time()
    res = run_bass_kernel_spmd(nc, in_maps, core_ids=list(range(CORES)))
    _PROGRAM_CACHE["device_wall_ns"] = int((_time.time() - _t0) * 1e9)
    if getattr(res, "exec_time_ns", None) is not None:
        _PROGRAM_CACHE["exec_time_ns"] = res.exec_time_ns
    cnt_a = np.concatenate(
        [res.results[c]["outa"].reshape(QA_PT, NCH_A, 128).sum(axis=1).reshape(-1)
         for c in range(CORES)]
    )
    cnt_b = np.concatenate(
        [res.results[c]["outb"].reshape(QB_PT, NCH_B, 128).sum(axis=1).reshape(-1)
         for c in range(CORES)]
    )
    flags_a = np.nonzero(cnt_a > 0.5)[0]
    # every stage-B row passes against itself (diagonal), so > 1 means a
    # real (off-diagonal) near-duplicate candidate
    flags_b = np.nonzero(cnt_b > 1.5)[0]

    return _host_resolve(b1, b2, s1, s2, l1, l2, wh, flags_a, flags_b)


def _host_resolve(b1, b2, s1, s2, l1, l2, wh, flags_a, flags_b):
    # --- exact greedy match on candidate rows only ---
    used = np.zeros(N2, dtype=bool)
    mboxes = b1.copy()
    mscores = s1.copy()
    merged_rows = []
    thr_match = np.float32(MATCH_IOU)
    for i in flags_a:
        iou = _iou_row(b1[i], b2)
        iou = np.where((~used) & (l2 == l1[i]), iou, np.float32(0.0))
        j = int(np.argmax(iou))
        if iou[j] >= thr_match:
            tot = s1[i] + s2[j]
            mboxes[i] = (b1[i] * s1[i] + b2[j] * s2[j]) / tot
            mscores[i] = tot
            used[j] = True
            merged_rows.append(int(i))

    boxes_all = np.concatenate([mboxes, b2], axis=0)
    scores_all = np.concatenate([mscores, s2], axis=0)
    labels_all = np.concatenate([l1, l2], axis=0)
    valid = np.concatenate([np.ones(N1, dtype=bool), ~used])

    key = np.where(valid, scores_all, np.float32(-1.0))
    order = np.argsort(-key, kind="stable")
    bs = boxes_all[order]
    ss = scores_all[order]
    ls = labels_all[order]
    vs = valid[order]
    inv = np.empty(NB, dtype=np.int64)
    inv[order] = np.arange(NB)

    # --- exact NMS edge rescue ---
    # device stage B ran on PRE-merge boxes: flagged rows cover all edges
    # between unmerged boxes (symmetric); merged rows are rescanned fully.
    r_unsorted = set(int(x) for x in flags_b) | set(merged_rows)
    thr_nms = np.float32(NMS_IOU)
    edges = {}
    for r in r_unsorted:
        p = int(inv[r])
        iou = _iou_row(bs[p], bs)
        hits = np.nonzero((iou >= thr_nms) & (ls == ls[p]))[0]
        for qq in hits:
            qq = int(qq)
            if qq == p:
                continue
            lo, hi = (p, qq) if p < qq else (qq, p)
            edges.setdefault(lo, set()).add(hi)

    # --- sequential suppression scan (only edge nodes matter) ---
    suppressed = ~vs
    keep = np.zeros(NB, dtype=bool)
    if edges:
        for i in range(NB):
            k = not suppressed[i]
            keep[i] = k
            if k and i in edges:
                for b_ in edges[i]:
                    suppressed[b_] = True
    else:
        keep = vs.copy()

    label_map = np.array([2, 1], dtype=np.int32)
    boxes_out = (bs * wh).astype(np.float32)
    labels_out = label_map[np.clip(ls, 0, 1)].astype(np.int32)
    return boxes_out, labels_out, ss.astype(np.float32), keep


# revision 9
# speedup vs baseline: 1.1221x; 1.0169x over previous
"""Ensemble detection fusion (weighted-boxes-fusion match + soft-NMS dedup)
for Trainium2, 8 NeuronCores.

Strategy: the O(N^2) work — 4096x2048 greedy-match IoU tests and the
6144x6144 NMS IoU tests — runs on-device as a conservative *filter*
(division-free IoU threshold test with safety margins), sharded row-wise
across the 8 cores.  Matches/suppressions at these thresholds (0.8 / 0.95
IoU) are extremely sparse, so the host then *rescues* only the flagged rows
with the exact reference arithmetic (fp32, matching op order) and resolves
the short sequential dependency chains (greedy 'used' set, NMS suppression
scan) on those few rows.

Device test per pair (q, t), fp16 pixel space, DVE-only ops chosen for
their fast perf modes (tensor_scalar 4x, tensor_tensor 2x; no
scalar_tensor_tensor which runs 1x):
    dxpk = min(TX2,qx2) + min(K-TX1, K-qx1)          # = dx + K
    dypk = min(TY2,qy2) + min(K-TY1, K-qy1)          # = dy + K
    p    = dxpk * max(dypk, 0)
    flag pair iff  p - TS_t >= QS_q
with TS/QS = c*(1-MU)*area, c = thr/(1+thr).  In exact arithmetic
inter >= c*(Aq+At) <=> iou >= thr; the +K (3 px) additive slack covers
fp16 coordinate rounding (~1px absolute on dx/dy) and MU covers relative
rounding, so the device flag set is a strict superset of the exact set.
False positives only cost a few extra host-rescued rows.

The NMS matrix is symmetric, so stage B sweeps only j >= 1024*floor(i/1024)
(block upper triangle); rows are assigned to cores round-robin (i mod 8)
so each core holds one 128-row tile per 1024-row block and the triangle
work is perfectly balanced.  Every unordered pair is swept by at least one
of its two rows, and the host rescue scans full rows of flagged indices.
"""

import numpy as np

N1, N2 = 4096, 2048
NB = N1 + N2
CORES = 8
YOLO_W = 0.5
FRCNN_W = 0.5
MATCH_IOU = 0.8
NMS_IOU = 0.95
MU = 0.02  # relative margin on the device filter
KPX = 3.0  # additive pixel slack on overlap widths

QA_PT = N1 // CORES // 128  # 4 query tiles of 128 rows per core (stage A)
QB_PT = NB // CORES // 128  # 6 query tiles per core (stage B)
CHUNK_A = 2048

# stage B chunk plan: (qtile k, target start, size); qtile k of core c holds
# global rows {1024*k + c + 8*m, m=0..127} and sweeps targets j >= 1024*k.
PLAN_B = [
    (0, 0, 3072), (0, 3072, 3072),
    (1, 1024, 3072), (1, 4096, 2048),
    (2, 2048, 3072), (2, 5120, 1024),
    (3, 3072, 3072),
    (4, 4096, 2048),
    (5, 5120, 1024),
]

# target blob layout (fp16): X2[NB] Y2[NB] NX1K[NB] NY1K[NB] SB[NB] SA[N2]
# where NX1K = KPX - x1, NY1K = KPX - y1.  Stage A uses the b2 tail
# (rows N1..NB) of the coordinate blocks plus the SA block.
TLEN = 5 * NB + N2

_PROGRAM_CACHE = {}


def _emit_combo(nc, mybir, X2, Y2, NX1K, NY1K, TS, q, pw, pc, out_ap, chunk):
    f16 = mybir.dt.float16
    f32 = mybir.dt.float32
    Alu = mybir.AluOpType
    nqx1k, nqy1k, qx2, qy2, qs = (q[:, k : k + 1] for k in range(5))
    m1x = pw.tile([128, chunk], f16, tag="m1")
    nc.vector.tensor_scalar(m1x[:, :], X2, qx2, None, Alu.min)
    a1x = pw.tile([128, chunk], f16, tag="a1")
    nc.vector.tensor_scalar(a1x[:, :], NX1K, nqx1k, None, Alu.min)
    dxpk = pw.tile([128, chunk], f16, tag="dp")
    nc.vector.tensor_tensor(dxpk[:, :], m1x[:, :], a1x[:, :], Alu.add)
    m1y = pw.tile([128, chunk], f16, tag="m1")
    nc.vector.tensor_scalar(m1y[:, :], Y2, qy2, None, Alu.min)
    a1y = pw.tile([128, chunk], f16, tag="a1")
    nc.vector.tensor_scalar(a1y[:, :], NY1K, nqy1k, None, Alu.min)
    dypk = pw.tile([128, chunk], f16, tag="dp2")
    nc.vector.tensor_tensor(dypk[:, :], m1y[:, :], a1y[:, :], Alu.add)
    dxk = pw.tile([128, chunk], f16, tag="dk")
    nc.vector.tensor_scalar(dxk[:, :], dxpk[:, :], 0.0, None, Alu.max)
    dyk = pw.tile([128, chunk], f16, tag="dk2")
    nc.vector.tensor_scalar(dyk[:, :], dypk[:, :], 0.0, None, Alu.max)
    p = pw.tile([128, chunk], f16, tag="p")
    nc.vector.tensor_tensor(p[:, :], dxk[:, :], dyk[:, :], Alu.mult)
    v = pw.tile([128, chunk], f16, tag="v")
    nc.vector.tensor_tensor(v[:, :], p[:, :], TS, Alu.subtract)
    g = pw.tile([128, chunk], f16, tag="g")
    cnt = pc.tile([128, 1], f32, tag="cnt")
    nc.vector.tensor_scalar(
        g[:, :], v[:, :], qs, None, Alu.is_ge, Alu.add, accum_out=cnt[:, :]
    )
    nc.sync.dma_start(out_ap, cnt[:, :])


def _build_program():
    import concourse.bacc as bacc
    import concourse.mybir as mybir
    from concourse import tile

    f16 = mybir.dt.float16
    f32 = mybir.dt.float32
    nc = bacc.Bacc(
        "TRN2", target_bir_lowering=False, debug=False, num_devices=CORES
    )
    qa = nc.dram_tensor("qa", [QA_PT, 128, 5], f32, kind="ExternalInput")
    qb = nc.dram_tensor("qb", [QB_PT, 128, 5], f32, kind="ExternalInput")
    tbl = nc.dram_tensor("tbl", [1, TLEN], f16, kind="ExternalInput")
    outa = nc.dram_tensor("outa", [QA_PT, 1, 128, 1], f32, kind="ExternalOutput")
    outb = nc.dram_tensor("outb", [len(PLAN_B), 128, 1], f32, kind="ExternalOutput")

    with tile.TileContext(nc) as tc:
        with (
            tc.tile_pool(name="tgt", bufs=1) as pt,
            tc.tile_pool(name="qs", bufs=3) as pq,
            tc.tile_pool(name="wk", bufs=2) as pw,
            tc.tile_pool(name="ct", bufs=4) as pc,
        ):
            tfull = pt.tile([128, TLEN], f16, tag="tfull")
            blk = 4096
            for off in range(0, TLEN, blk):
                n = min(blk, TLEN - off)
                trow = pq.tile([1, blk], f16, tag="trow")
                nc.sync.dma_start(trow[:1, :n], tbl.ap()[:, off : off + n])
                nc.gpsimd.partition_broadcast(
                    tfull[:, off : off + n], trow[0:1, :n]
                )
            X2 = tfull[:, 0 * NB : 1 * NB]
            Y2 = tfull[:, 1 * NB : 2 * NB]
            NX1K = tfull[:, 2 * NB : 3 * NB]
            NY1K = tfull[:, 3 * NB : 4 * NB]
            SB = tfull[:, 4 * NB : 5 * NB]
            SA = tfull[:, 5 * NB : 5 * NB + N2]

            # stage A: queries = yolo shard, targets = frcnn boxes
            # (= rows N1..NB of the coordinate blocks)
            for qt in range(QA_PT):
                q = pq.tile([128, 5], f32, tag="q")
                nc.sync.dma_start(q[:, :], qa.ap()[qt, :, :])
                sl = slice(N1, N1 + CHUNK_A)
                _emit_combo(
                    nc, mybir,
                    X2[:, sl], Y2[:, sl], NX1K[:, sl], NY1K[:, sl],
                    SA[:, 0:CHUNK_A],
                    q, pw, pc, outa.ap()[qt, 0, :, :], CHUNK_A,
                )
            # stage B: block-upper-triangle sweep
            qtiles_b = {}
            for idx, (k, start, size) in enumerate(PLAN_B):
                if k not in qtiles_b:
                    q = pq.tile([128, 5], f32, tag="q")
                    nc.sync.dma_start(q[:, :], qb.ap()[k, :, :])
                    qtiles_b[k] = q
                q = qtiles_b[k]
                sl = slice(start, start + size)
                _emit_combo(
                    nc, mybir,
                    X2[:, sl], Y2[:, sl], NX1K[:, sl], NY1K[:, sl], SB[:, sl],
                    q, pw, pc, outb.ap()[idx, :, :], size,
                )
    nc.compile()
    return nc


def get_program():
    if "nc" not in _PROGRAM_CACHE:
        _PROGRAM_CACHE["nc"] = _build_program()
    return _PROGRAM_CACHE["nc"]


def _iou_row(box, B):
    # Exact replica of reference _iou_one_vs_many op order (fp32, IEEE).
    x1 = np.maximum(box[0], B[:, 0])
    y1 = np.maximum(box[1], B[:, 1])
    x2 = np.minimum(box[2], B[:, 2])
    y2 = np.minimum(box[3], B[:, 3])
    inter = np.maximum(x2 - x1, np.float32(0.0)) * np.maximum(y2 - y1, np.float32(0.0))
    a1 = (box[2] - box[0]) * (box[3] - box[1])
    a2 = (B[:, 2] - B[:, 0]) * (B[:, 3] - B[:, 1])
    return inter / (a1 + a2 - inter)


def _stage_b_rows(core):
    """Global row indices handled by `core`, tile-major: [k, m] -> row."""
    k = np.arange(QB_PT)[:, None]
    m = np.arange(128)[None, :]
    return 1024 * k + core + 8 * m


def make_device_inputs(p1, p2, pall):
    """p1/p2/pall: pixel-space fp32 box arrays."""
    a1 = (p1[:, 2] - p1[:, 0]) * (p1[:, 3] - p1[:, 1])
    a2 = (p2[:, 2] - p2[:, 0]) * (p2[:, 3] - p2[:, 1])
    aall = np.concatenate([a1, a2])
    cA = np.float32((1.0 - MU) * MATCH_IOU / (1.0 + MATCH_IOU))
    cB = np.float32((1.0 - MU) * NMS_IOU / (1.0 + NMS_IOU))
    kpx = np.float32(KPX)

    def qpack(boxes, scaled_area):
        return np.stack(
            [kpx - boxes[:, 0], kpx - boxes[:, 1], boxes[:, 2], boxes[:, 3],
             scaled_area], axis=1
        ).astype(np.float32)

    qa_all = qpack(p1, cA * a1)
    qb_all = qpack(pall, cB * aall)
    tbl = np.concatenate(
        [pall[:, 2], pall[:, 3], kpx - pall[:, 0], kpx - pall[:, 1],
         cB * aall, cA * a2]
    ).astype(np.float16).reshape(1, -1)
    tbl = np.ascontiguousarray(tbl)
    in_maps = []
    ra = N1 // CORES
    for c in range(CORES):
        rows_b = _stage_b_rows(c).reshape(-1)
        in_maps.append(
            {
                "qa": np.ascontiguousarray(
                    qa_all[c * ra : (c + 1) * ra].reshape(QA_PT, 128, 5)
                ),
                "qb": np.ascontiguousarray(
                    qb_all[rows_b].reshape(QB_PT, 128, 5)
                ),
                "tbl": tbl,
            }
        )
    return in_maps


def kernel(**inputs):
    yolo_boxes = np.asarray(inputs["yolo_boxes"], dtype=np.float32)
    yolo_scores = np.asarray(inputs["yolo_scores"], dtype=np.float32)
    yolo_labels = np.asarray(inputs["yolo_labels"], dtype=np.int32)
    frcnn_boxes = np.asarray(inputs["frcnn_boxes"], dtype=np.float32)
    frcnn_scores = np.asarray(inputs["frcnn_scores"], dtype=np.float32)
    frcnn_labels = np.asarray(inputs["frcnn_labels"], dtype=np.int32)
    h = float(np.asarray(inputs["h"]))
    w = float(np.asarray(inputs["w"]))

    wh = np.array([w, h, w, h], dtype=np.float32)
    b1 = (yolo_boxes / wh).astype(np.float32)
    b2 = (frcnn_boxes / wh).astype(np.float32)
    s1 = (yolo_scores * np.float32(YOLO_W)).astype(np.float32)
    s2 = (frcnn_scores * np.float32(FRCNN_W)).astype(np.float32)
    l1, l2 = yolo_labels, frcnn_labels

    # --- device filter: 8-core SPMD sweep over both IoU-test matrices ---
    from concourse.bass_utils import run_bass_kernel_spmd

    nc = get_program()
    pall = np.concatenate([yolo_boxes, frcnn_boxes], axis=0)
    in_maps = make_device_inputs(yolo_boxes, frcnn_boxes, pall)
    import time as _time

    _t0 = _time.time()
    res = run_bass_kernel_spmd(nc, in_maps, core_ids=list(range(CORES)))
    _PROGRAM_CACHE["device_wall_ns"] = int((_time.time() - _t0) * 1e9)
    if getattr(res, "exec_time_ns", None) is not None:
        _PROGRAM_CACHE["exec_time_ns"] = res.exec_time_ns

    cnt_a = np.concatenate(
        [res.results[c]["outa"].reshape(-1) for c in range(CORES)]
    )
    flags_a = np.nonzero(cnt_a > 0.5)[0]

    cnt_b = np.zeros(NB, dtype=np.float64)
    for c in range(CORES):
        rows_b = _stage_b_rows(c)  # [QB_PT, 128]
        ob = res.results[c]["outb"].reshape(len(PLAN_B), 128)
        for idx, (k, _start, _size) in enumerate(PLAN_B):
            cnt_b[rows_b[k]] += ob[idx]
    # every stage-B row sweeps itself (diagonal), so > 1 means a real
    # (off-diagonal) near-duplicate candidate
    flags_b = np.nonzero(cnt_b > 1.5)[0]

    return _host_resolve(b1, b2, s1, s2, l1, l2, wh, flags_a, flags_b)


def _host_resolve(b1, b2, s1, s2, l1, l2, wh, flags_a, flags_b):
    # --- exact greedy match on candidate rows only ---
    used = np.zeros(N2, dtype=bool)
    mboxes = b1.copy()
    mscores = s1.copy()
    merged_rows = []
    thr_match = np.float32(MATCH_IOU)
    for i in flags_a:
        iou = _iou_row(b1[i], b2)
        iou = np.where((~used) & (l2 == l1[i]), iou, np.float32(0.0))
        j = int(np.argmax(iou))
        if iou[j] >= thr_match:
            tot = s1[i] + s2[j]
            mboxes[i] = (b1[i] * s1[i] + b2[j] * s2[j]) / tot
            mscores[i] = tot
            used[j] = True
            merged_rows.append(int(i))

    boxes_all = np.concatenate([mboxes, b2], axis=0)
    scores_all = np.concatenate([mscores, s2], axis=0)
    labels_all = np.concatenate([l1, l2], axis=0)
    valid = np.concatenate([np.ones(N1, dtype=bool), ~used])

    key = np.where(valid, scores_all, np.float32(-1.0))
    order = np.argsort(-key, kind="stable")
    bs = boxes_all[order]
    ss = scores_all[order]
    ls = labels_all[order]
    vs = valid[order]
    inv = np.empty(NB, dtype=np.int64)
    inv[order] = np.arange(NB)

    # --- exact NMS edge rescue ---
    # device stage B ran on PRE-merge boxes: flagged rows cover all edges
    # between unmerged boxes (symmetric); merged rows are rescanned fully.
    r_unsorted = set(int(x) for x in flags_b) | set(merged_rows)
    thr_nms = np.float32(NMS_IOU)
    edges = {}
    for r in r_unsorted:
        p = int(inv[r])
        iou = _iou_row(bs[p], bs)
        hits = np.nonzero((iou >= thr_nms) & (ls == ls[p]))[0]
        for qq in hits:
            qq = int(qq)
            if qq == p:
                continue
            lo, hi = (p, qq) if p < qq else (qq, p)
            edges.setdefault(lo, set()).add(hi)

    # --- sequential suppression scan (only edge nodes matter) ---
    suppressed = ~vs
    keep = np.zeros(NB, dtype=bool)
    if edges:
        for i in range(NB):
            k = not suppressed[i]
            keep[i] = k
            if k and i in edges:
                for b_ in edges[i]:
                    suppressed[b_] = True
    else:
        keep = vs.copy()

    label_map = np.array([2, 1], dtype=np.int32)
    boxes_out = (bs * wh).astype(np.float32)
    labels_out = label_map[np.clip(ls, 0, 1)].astype(np.int32)
    return boxes_out, labels_out, ss.astype(np.float32), keep
